# revision 1
# baseline (speedup 1.0000x reference)
"""AttentionContext kernel for Trainium2, data-parallel over batch on 8 cores.

Reference computation (B=64, T=2048, D=512 everywhere):
    phi_s = s @ phi_w.T + phi_b                  # [B, D]
    psi_h = einsum('bth,ah->bta', h, psi_w) + psi_b
    e     = einsum('ba,bta->bt', phi_s, psi_h)   # [B, T]
    alpha = softmax(e, axis=-1)
    c     = alpha * h.sum(-1)                    # [B, T]

Algebraic restructuring used here:
    e[b,t] = (phi_s[b] @ psi_w) . h[b,t] + const(b)   (const dropped: softmax
    is shift-invariant).  w = s @ (phi_w.T @ psi_w) + phi_b @ psi_w; both
    weight contractions run over the leading (partition-natural) dim, so no
    weight transposes are needed.

v4 structure (per core, 8 batches):
  - sync DMA queue carries only: s/phi_b/weight chunks, then all h
    super-tiles back-to-back, then one final 64KB output store.
  - stage-0 critical path trimmed: PSUM->SBUF copies run on ScalarE (DVE's
    stream starts directly with streaming work), M_c accumulates ac-outer
    across 4 PSUM banks, phi_b folds into the w-chain as a replicated-rhs
    matmul, and the per-batch w broadcasts run as one PE row-select
    matmul each (batch 0 first, to unblock the first STT).
  - per h tile [128t, 512d]: e via DVE scalar_tensor_tensor accumulate;
    hsum split between ScalarE activation-accumulate and DVE grouped
    tensor_reduce for load balance.
  - softmax: exp uses per-partition column-max as bias (nothing
    cross-partition gates the exp).  The cross-partition correction
    alpha = exp(e - cm[p]) * exp(cm[p] - M) / sum runs via two GpSimd
    partition_all_reduce ops, software-pipelined across batches so no
    engine idles waiting on it.
"""

import numpy as np

import concourse.bass as bass
import concourse.bacc as bacc
import concourse.tile as tile
from concourse import mybir
from concourse import bass_isa
from concourse import bass_utils
from concourse.masks import make_identity

FP = mybir.dt.float32
ALU = mybir.AluOpType
AF = mybir.ActivationFunctionType
RED = bass_isa.ReduceOp

N_CORES = 8
B_LOC = 8          # batches per core
T = 2048
D = 512
P = 128
KC = D // P        # 4 contraction chunks of 128
TI = T // P        # 16 t-tiles per batch
SUP = 8            # t-tiles per DMA super-tile
NSUP = TI // SUP   # 2 super-tiles per batch
NST = B_LOC * NSUP
HBUFS = 9          # h super-tile buffers in flight (9 * 2MB = 18MB SBUF)

# trailing t-tiles per super-tile whose hsum runs as one grouped DVE
# tensor_reduce (the rest go to ScalarE activation-accumulate)
R_PER_ST = [8, 8, 2, 2, 1, 1, 1, 1, 1, 1, 1] + [0] * (NST - 11)


def _emit(nc, tc, variant="full"):
    s = nc.dram_tensor("s", [B_LOC, D], FP, kind="ExternalInput").ap()
    h = nc.dram_tensor("h", [B_LOC, T, D], FP, kind="ExternalInput").ap()
    phi_w = nc.dram_tensor("phi_w", [D, D], FP, kind="ExternalInput").ap()
    phi_b = nc.dram_tensor("phi_b", [D], FP, kind="ExternalInput").ap()
    psi_w = nc.dram_tensor("psi_w", [D, D], FP, kind="ExternalInput").ap()
    c_out = nc.dram_tensor("c", [B_LOC, T], FP, kind="ExternalOutput").ap()

    with (
        tc.tile_pool(name="consts", bufs=1) as consts,
        tc.tile_pool(name="hpool", bufs=HBUFS) as hpool,
    ):
        # ---------------- input DMAs (sync queue, in priority order) -------
        s_sb = consts.tile([B_LOC, D], FP)
        nc.sync.dma_start(out=s_sb, in_=s)
        phi_b_sb = consts.tile([P, KC], FP)      # [a % 128, a // 128]
        nc.sync.dma_start(out=phi_b_sb, in_=phi_b.rearrange("(ac p) -> p ac", p=P))
        phi_w_sb = consts.tile([P, KC, D], FP)   # [a % 128, a // 128, k]
        psi_w_sb = consts.tile([P, KC, D], FP)   # [a % 128, a // 128, m]
        for ac in range(KC):
            nc.sync.dma_start(
                out=psi_w_sb[:, ac, :], in_=psi_w[ac * P : (ac + 1) * P, :]
            )
            nc.sync.dma_start(
                out=phi_w_sb[:, ac, :], in_=phi_w[ac * P : (ac + 1) * P, :]
            )

        ident = consts.tile([P, P], FP)
        make_identity(nc, ident)
        ones_1x128 = consts.tile([1, P], FP)
        nc.vector.memset(ones_1x128, 1.0)

        # Warm the ACT exp table set early so the ~2.7us load overlaps.
        tiny = consts.tile([1, 1], FP)
        nc.vector.memset(tiny, 0.0)
        nc.scalar.activation(out=tiny, in_=tiny, func=AF.Exp)

        # phi_b replicated along a free b-axis: [P, KC, B_LOC]
        phib_rep = consts.tile([P, KC, B_LOC], FP)
        pb = phi_b_sb[:, :]
        nc.vector.tensor_copy(
            out=phib_rep,
            in_=bass.AP(
                tensor=pb.tensor,
                offset=pb.offset,
                ap=[[pb.ap[0][0], P], [pb.ap[-1][0], KC], [0, B_LOC]],
            ),
        )

        mc_sb = consts.tile([P, KC, D], FP)      # M_c[k, m], k = kc*128 + p
        sT_sb = consts.tile([P, KC, B_LOC], FP)  # s.T[k, b]
        w_sb = consts.tile([B_LOC, D], FP)       # w[b, m]
        w_bc = consts.tile([P, B_LOC, D], FP)    # w[b] broadcast down parts
        e_all = consts.tile([P, P], FP)          # e[t%128, b*16 + ti]
        hs_all = consts.tile([P, P], FP)         # hsum, same layout
        exp_all = consts.tile([P, P], FP)        # exp(e - colmax), same layout
        c_acc = consts.tile([P, B_LOC, TI], FP)  # staged outputs

        with (
            tc.tile_pool(name="psA", bufs=1, space="PSUM") as psA,
            tc.tile_pool(name="psB", bufs=1, space="PSUM") as psB,
        ):
            # s.T chunks via PE transpose (s arrives first); all four land in
            # one PSUM tile so a single ScalarE copy moves them out.
            st_ps = psB.tile([P, KC, B_LOC], FP, tag="st_ps")
            for kc in range(KC):
                nc.tensor.transpose(
                    st_ps[:, kc, :],
                    in_=s_sb[:, kc * P : (kc + 1) * P],
                    identity=ident[:B_LOC, :B_LOC],
                )
            nc.scalar.copy(out=sT_sb, in_=st_ps)

            # M_c[k, m] = sum_a phi_w[a, k] * psi_w[a, m]; ac-outer so the
            # PE chain starts as soon as the first weight chunks land.
            mc_ps = [
                psA.tile([P, D], FP, tag=f"mc{kc}", name=f"mc_ps{kc}")
                for kc in range(KC)
            ]
            # w[b, m] = sum_ac phib_rep[a, b] * psi_w[a, m]  (v-term,
            # interleaved with M_c so only the sT@M_c tail trails M_c)
            w_ps = psB.tile([B_LOC, D], FP, tag="w_ps")
            for ac in range(KC):
                nc.tensor.matmul(
                    w_ps,
                    lhsT=phib_rep[:, ac, :],
                    rhs=psi_w_sb[:, ac, :],
                    start=(ac == 0),
                    stop=False,
                )
                for kc in range(KC):
                    nc.tensor.matmul(
                        mc_ps[kc],
                        lhsT=phi_w_sb[:, ac, kc * P : (kc + 1) * P],
                        rhs=psi_w_sb[:, ac, :],
                        start=(ac == 0),
                        stop=(ac == KC - 1),
                    )
                if ac == KC - 1:
                    for kc in range(KC):
                        nc.scalar.copy(out=mc_sb[:, kc, :], in_=mc_ps[kc])

            for kc in range(KC):
                nc.tensor.matmul(
                    w_ps,
                    lhsT=sT_sb[:, kc, :],
                    rhs=mc_sb[:, kc, :],
                    start=False,
                    stop=(kc == KC - 1),
                )
            nc.scalar.copy(out=w_sb, in_=w_ps)

            # broadcast each w row down the partitions in one PE op per b:
            # out[p, m] = sum_q rowsel[q, b, p] * w[q, m] with rowsel row b = 1
            rowsel = consts.tile([B_LOC, B_LOC, P], FP)
            ident_bc = bass.AP(
                tensor=ident.tensor,
                offset=ident.offset,
                ap=[[ident.ap[0][0], B_LOC], [ident.ap[-1][0], B_LOC], [0, P]],
            )
            nc.vector.tensor_copy(out=rowsel, in_=ident_bc)
            for b in range(B_LOC):
                bc_ps = psB.tile([P, D], FP, tag=f"bc{b % 2}", name=f"bc{b}")
                nc.tensor.matmul(bc_ps, lhsT=rowsel[:, b, :], rhs=w_sb)
                nc.scalar.copy(out=w_bc[:, b, :], in_=bc_ps)

        if variant == "s0":
            nc.sync.dma_start(out=c_out[:, :D], in_=w_bc[:B_LOC, 0, :])
            return

        # ---------------- stream h; softmax pipelined across batches -------
        with (
            tc.tile_pool(name="junk", bufs=2) as junk,
            tc.tile_pool(name="small", bufs=4) as small,
        ):
            state = {}

            def part_a_dve(b):
                cols = slice(b * TI, (b + 1) * TI)
                colmax = small.tile([P, 1], FP, tag="colmax")
                nc.vector.tensor_reduce(
                    out=colmax, in_=e_all[:, cols], axis=mybir.AxisListType.X,
                    op=ALU.max,
                )
                nbc = small.tile([P, 1], FP, tag="nbc")
                nc.vector.tensor_scalar_mul(out=nbc, in0=colmax, scalar1=-1.0)
                mb = small.tile([P, 1], FP, tag="mb")
                nc.gpsimd.partition_all_reduce(
                    out_ap=mb, in_ap=colmax, channels=P, reduce_op=RED.max
                )
                state[b] = (colmax, nbc, mb)

            def part_a_act(b):
                colmax, nbc, mb = state[b]
                cols = slice(b * TI, (b + 1) * TI)
                pscol = small.tile([P, 1], FP, tag="pscol")
                nc.scalar.activation(
                    out=exp_all[:, cols],
                    in_=e_all[:, cols],
                    func=AF.Exp,
                    bias=nbc,
                    scale=1.0,
                    accum_out=pscol,
                )
                state[b] = (colmax, mb, pscol)

            def part_b1(b):
                colmax, mb, pscol = state[b]
                dcm = small.tile([P, 1], FP, tag="dcm")
                nc.vector.tensor_tensor(
                    out=dcm, in0=colmax, in1=mb, op=ALU.subtract
                )
                tcor = small.tile([P, 1], FP, tag="tcor")
                nc.scalar.activation(out=tcor, in_=dcm, func=AF.Exp)
                sv = small.tile([P, 1], FP, tag="sv")
                nc.vector.tensor_tensor(out=sv, in0=pscol, in1=tcor, op=ALU.mult)
                sb = small.tile([P, 1], FP, tag="sb")
                nc.gpsimd.partition_all_reduce(
                    out_ap=sb, in_ap=sv, channels=P, reduce_op=RED.add
                )
                state[b] = (tcor, sb)

            def part_b2(b):
                tcor, sb = state.pop(b)
                cols = slice(b * TI, (b + 1) * TI)
                rs = small.tile([P, 1], FP, tag="rs")
                nc.vector.reciprocal(out=rs, in_=sb)
                cbuf = small.tile([P, TI], FP, tag="cbuf")
                nc.vector.scalar_tensor_tensor(
                    out=cbuf,
                    in0=exp_all[:, cols],
                    scalar=tcor,
                    in1=hs_all[:, cols],
                    op0=ALU.mult,
                    op1=ALU.mult,
                )
                nc.vector.tensor_scalar_mul(
                    out=c_acc[:, b, :], in0=cbuf, scalar1=rs
                )
                # store batch b: t = j*SUP*P + p*SUP + jt
                nc.sync.dma_start(
                    out=c_out[b, :].rearrange("(j p jt) -> p j jt", p=P, jt=SUP),
                    in_=c_acc[:, b, :],
                )

            for st in range(NST):
                b, j = divmod(st, NSUP)
                r_dve = R_PER_ST[st]
                ht = hpool.tile([P, SUP, D], FP, tag="ht")
                nc.sync.dma_start(
                    out=ht,
                    in_=h[b, j * SUP * P : (j + 1) * SUP * P, :].rearrange(
                        "(p jt) d -> p jt d", p=P
                    ),
                )
                col0 = b * TI + j * SUP
                if r_dve > 0:
                    nc.vector.tensor_reduce(
                        out=hs_all[:, col0 + SUP - r_dve : col0 + SUP],
                        in_=ht[:, SUP - r_dve :, :],
                        axis=mybir.AxisListType.X,
                        op=ALU.add,
                    )
                for jt in range(SUP):
                    col = col0 + jt
                    jd = junk.tile([P, D], FP, tag="jd")
                    # fused (h * w) multiply + free-dim sum on DVE.
                    nc.vector.scalar_tensor_tensor(
                        out=jd,
                        in0=ht[:, jt, :],
                        scalar=1.0,
                        in1=w_bc[:, b, :],
                        op0=ALU.mult,
                        op1=ALU.mult,
                        accum_out=e_all[:, col : col + 1],
                    )
                    if jt >= SUP - r_dve:
                        continue  # hsum handled by the grouped reduce
                    ja = junk.tile([P, D], FP, tag="ja")
                    nc.scalar.activation(
                        out=ja,
                        in_=ht[:, jt, :],
                        func=AF.Copy,
                        accum_out=hs_all[:, col : col + 1],
                    )
                if variant == "s1":
                    continue
                # pipelined softmax stages, each one supertile apart
                if j == NSUP - 1:
                    part_a_dve(b)
                    if b >= 1:
                        part_b1(b - 1)
                else:
                    if b >= 1:
                        part_a_act(b - 1)
                    if b >= 2:
                        part_b2(b - 2)

            if variant == "s1":
                nc.sync.dma_start(out=c_out[:, :P], in_=e_all)
                nc.sync.dma_start(out=c_out[:, P : 2 * P], in_=hs_all)
                return

            part_a_act(B_LOC - 1)
            part_b1(B_LOC - 1)
            part_b2(B_LOC - 2)
            part_b2(B_LOC - 1)


_CACHE = {}


def _build():
    if "nc" not in _CACHE:
        nc = bacc.Bacc(
            "TRN2", target_bir_lowering=False, debug=False, num_devices=N_CORES
        )
        with tile.TileContext(nc) as tc:
            _emit(nc, tc)
        nc.compile()
        _CACHE["nc"] = nc
    return _CACHE["nc"]


def kernel(s, h, phi_w, phi_b, psi_w, psi_b=None, **_unused):
    s = np.ascontiguousarray(np.asarray(s, dtype=np.float32))
    h = np.ascontiguousarray(np.asarray(h, dtype=np.float32))
    phi_w = np.ascontiguousarray(np.asarray(phi_w, dtype=np.float32))
    phi_b = np.ascontiguousarray(np.asarray(phi_b, dtype=np.float32))
    psi_w = np.ascontiguousarray(np.asarray(psi_w, dtype=np.float32))

    nc = _build()
    in_maps = [
        {
            "s": s[i * B_LOC : (i + 1) * B_LOC],
            "h": h[i * B_LOC : (i + 1) * B_LOC],
            "phi_w": phi_w,
            "phi_b": phi_b,
            "psi_w": psi_w,
        }
        for i in range(N_CORES)
    ]
    res = bass_utils.run_bass_kernel_spmd(nc, in_maps, core_ids=list(range(N_CORES)))
    return np.concatenate(
        [res.results[i]["c"] for i in range(N_CORES)], axis=0
    ).astype(np.float32)

